# revision 1
# baseline (speedup 1.0000x reference)
"""v2.5: valid-only [128,1] indirect gathers, length-sorted nodes, raw Block.

Nodes are sorted by degree (desc) per core so each 128-node tile only
gathers max-degree-in-tile neighbor columns (~half the slots are padding
in the unsorted layout). Raw Bass Block avoids per-call Tile sync cost.
"""
import os
import sys

for _p in ("/opt/trn_rl_repo", "/opt/pypackages"):
    if _p not in sys.path and os.path.isdir(_p):
        sys.path.append(_p)

import numpy as np

NUM_AUTHOR = 131072
D = 128
N_NODES = 32768
G = 32
NCORES = 8
NPC = N_NODES // NCORES   # 4096
P = 128
TILES = NPC // P          # 32
ZERO_ROW = NUM_AUTHOR

_CACHE = {}
LAST_RESULT = None


def _tile_maxlens(lengths):
    """Per-core sort order and per-tile gather column counts (compile-time)."""
    lengths = np.asarray(lengths).reshape(NCORES, NPC)
    orders, tlens = [], []
    for c in range(NCORES):
        order = np.argsort(-lengths[c], kind="stable")
        lens_sorted = lengths[c][order]
        lt = [max(int(lens_sorted[t * P]), 1) for t in range(TILES)]
        orders.append(order)
        tlens.append(lt)
    return orders, tlens


def _build_program(tile_lens):
    """tile_lens: [TILES] ints — max over cores of each tile's column count
    (SPMD: one program for all cores)."""
    from concourse import bacc, bass, mybir

    nc = bacc.Bacc("TRN2", target_bir_lowering=False, debug=False,
                   enable_asserts=False, num_devices=NCORES)
    dt = mybir.dt
    ctotal = sum(tile_lens)
    a2e = nc.dram_tensor("a2e", [NUM_AUTHOR + 1, D], dt.float32, kind="ExternalInput")
    idx = nc.dram_tensor("idx", [P, ctotal], dt.int32, kind="ExternalInput")
    scl = nc.dram_tensor("scl", [P, TILES], dt.float32, kind="ExternalInput")
    out = nc.dram_tensor("out", [NPC, D], dt.float32, kind="ExternalOutput")

    csum = [0]
    for L in tile_lens:
        csum.append(csum[-1] + L)

    with (
        nc.Block() as block,
        nc.sbuf_tensor("idx_sb", [P, ctotal], dt.int32) as idx_sb,
        nc.sbuf_tensor("scl_sb", [P, TILES], dt.float32) as scl_sb,
        nc.sbuf_tensor("g0", [P, G * D], dt.float32) as g0,
        nc.sbuf_tensor("g1", [P, G * D], dt.float32) as g1,
        nc.sbuf_tensor("r0", [P, D], dt.float32) as r0,
        nc.sbuf_tensor("r1", [P, D], dt.float32) as r1,
        nc.semaphore("iosem") as iosem,
        nc.semaphore("dsem0") as dsem0,
        nc.semaphore("dsem1") as dsem1,
        nc.semaphore("rsem") as rsem,
        nc.semaphore("esem") as esem,
        nc.semaphore("wsem0") as wsem0,
        nc.semaphore("wsem1") as wsem1,
    ):
        gbuf = [g0, g1]
        rbuf = [r0, r1]
        dsem = [dsem0, dsem1]
        wsem = [wsem0, wsem1]
        # cumulative gather-call counts per tile parity
        cumpar = {0: [], 1: []}
        tot = {0: 0, 1: 0}
        for t, L in enumerate(tile_lens):
            tot[t % 2] += L
            cumpar[t % 2].append(tot[t % 2])

        @block.sync
        def _(sync):
            sync.dma_start(out=idx_sb[:], in_=idx[:]).then_inc(iosem, 16)
            sync.dma_start(out=scl_sb[:], in_=scl[:]).then_inc(iosem, 16)
            for t in range(TILES):
                sync.wait_ge(rsem, t + 1)
                sync.dma_start(
                    out=out[t * P:(t + 1) * P, :], in_=rbuf[t % 2][:]
                ).then_inc(wsem[t % 2], 16)
            sync.wait_ge(wsem0, 16 * (TILES // 2))
            sync.wait_ge(wsem1, 16 * (TILES // 2))

        @block.gpsimd
        def _(gpsimd):
            gpsimd.wait_ge(iosem, 32)  # idx + scl loaded
            for t in range(TILES):
                if t >= 2:
                    gpsimd.wait_ge(rsem, t - 1)  # g[t%2] free after reduce t-2
                for j in range(tile_lens[t]):
                    c = csum[t] + j
                    gpsimd.indirect_dma_start(
                        out=gbuf[t % 2][:, j * D:(j + 1) * D],
                        out_offset=None,
                        in_=a2e[:],
                        in_offset=bass.IndirectOffsetOnAxis(
                            ap=idx_sb[:, c:c + 1], axis=0,
                        ),
                    ).then_inc(dsem[t % 2], 16)

        @block.vector
        def _(vector):
            vector.wait_ge(iosem, 32)  # scl loaded
            for t in range(TILES):
                vector.wait_ge(dsem[t % 2], 16 * cumpar[t % 2][t // 2])
                if t >= 2:
                    vector.wait_ge(wsem[t % 2], 16 * (t // 2))  # r[t%2] free
                L = tile_lens[t]
                gv = (gbuf[t % 2][:]
                      .rearrange("p (g d) -> p d g", g=G, d=D)[:, :, 0:L])
                vector.tensor_reduce(
                    out=rbuf[t % 2][:], in_=gv,
                    axis=mybir.AxisListType.X, op=mybir.AluOpType.add,
                ).then_inc(esem, 1)
                vector.wait_ge(esem, t + 1)
                sv = scl_sb[:, t:t + 1].broadcast_to([P, D])
                vector.tensor_tensor(
                    out=rbuf[t % 2][:], in0=rbuf[t % 2][:], in1=sv,
                    op=mybir.AluOpType.mult,
                ).then_inc(rsem, 1)

    nc.compile()
    return nc


def _prep_inputs(neighbors, lengths, a2e, orders, tile_lens):
    neighbors = np.asarray(neighbors).reshape(NCORES, NPC, G)
    lengths = np.asarray(lengths).reshape(NCORES, NPC)
    a2e = np.asarray(a2e, dtype=np.float32)
    ctotal = sum(tile_lens)

    idx_dram = np.full((NCORES, P, ctotal), ZERO_ROW, dtype=np.int32)
    scl_dram = np.zeros((NCORES, P, TILES), dtype=np.float32)
    for c in range(NCORES):
        order = orders[c]
        nb = neighbors[c][order]          # [NPC, G] sorted
        ln = lengths[c][order]            # [NPC]
        mask = np.arange(G)[None, :] < ln[:, None]
        nbc = np.where(mask, nb, ZERO_ROW).astype(np.int32)
        inv = np.where(ln > 0, 1.0 / np.maximum(ln, 1), 0.0).astype(np.float32)
        off = 0
        for t in range(TILES):
            L = tile_lens[t]
            idx_dram[c, :, off:off + L] = nbc[t * P:(t + 1) * P, :L]
            scl_dram[c, :, t] = inv[t * P:(t + 1) * P]
            off += L
    a2e_pad = np.concatenate([a2e, np.zeros((1, D), np.float32)], axis=0)
    return idx_dram, scl_dram, a2e_pad


def _install_ntff_hook_shim():
    import types
    if "antenv.axon_hooks" in sys.modules:
        return
    from trn_agent_boot.trn_boot import _ntff_profile_via_ctypes
    hook = _ntff_profile_via_ctypes("/opt/axon/libaxon_pjrt.so")
    mod = types.ModuleType("antenv.axon_hooks")
    mod._hook = hook
    mod.get_axon_ntff_profile_hook = lambda: mod._hook
    mod.set_axon_ntff_profile_hook = lambda h: setattr(mod, "_hook", h)
    sys.modules["antenv.axon_hooks"] = mod


def kernel(node, neighbors, lengths, a2e, _trace=False):
    global LAST_RESULT
    from concourse.bass_utils import run_bass_kernel_spmd

    if _trace:
        try:
            _install_ntff_hook_shim()
            import concourse.bass_utils as _bu
            _bu.upload_artifacts = lambda tmpdir: f"local://{tmpdir}"
        except Exception as e:
            print(f"ntff hook shim failed ({e}); running without trace")
            _trace = False

    orders, percore_lens = _tile_maxlens(lengths)
    tile_lens = [max(percore_lens[c][t] for c in range(NCORES))
                 for t in range(TILES)]
    key = tuple(tile_lens)
    if _CACHE.get("key") != key:
        _CACHE["nc"] = _build_program(tile_lens)
        _CACHE["key"] = key
    nc = _CACHE["nc"]

    idx_dram, scl_dram, a2e_pad = _prep_inputs(
        neighbors, lengths, a2e, orders, tile_lens)
    in_maps = [
        {
            "a2e": np.ascontiguousarray(a2e_pad),
            "idx": np.ascontiguousarray(idx_dram[c]),
            "scl": np.ascontiguousarray(scl_dram[c]),
        }
        for c in range(NCORES)
    ]
    res = run_bass_kernel_spmd(nc, in_maps, list(range(NCORES)), trace=_trace)
    LAST_RESULT = res

    final = np.empty((N_NODES, D), dtype=np.float32)
    for c in range(NCORES):
        block = final[c * NPC:(c + 1) * NPC]
        block[orders[c]] = res.results[c]["out"]
    return final



# revision 8
# speedup vs baseline: 3.7578x; 3.7578x over previous
"""v4: packed bf16 dma_gather + PE one-hot segment-sum.

Per core (4096 nodes, table replicated): nodes sorted by degree desc into 32
tiles of 128; tiles dealt round-robin into 4 groups (balanced bytes). Per
(group, table-chunk) the valid (idx, node) pairs of its 8 tiles are packed
into a contiguous stream (idx-sorted per tile, padded to x128, pad id=255)
and fetched with ONE dma_gather (int16 local idx, 16-partition wrap).
Gathered slot k lands at partition k%128, block k//128 — each 128-slot block
is a ready matmul rhs [k=128, d=128]. DVE builds the S^T one-hot [k, node]
per tile via is_equal(ids, iota); PE accumulates out[node, d] += S^T.T @ G
into PSUM; ACT copies PSUM->SBUF scaled by 1/len; sync stores per tile.

Why: the baseline's 528 indirect_dma_start/core each pay ~1us SWDGE fixed
cost (~570us serial descriptor generation). dma_gather amortizes it
(~num_idxs/16 descriptors, 16 calls/core) and bf16 halves HBM bytes. The
block structure is the max over cores so one SPMD program serves all 8.
"""
import os
import sys

for _p in ("/opt/trn_rl_repo", "/opt/pypackages"):
    if _p not in sys.path and os.path.isdir(_p):
        sys.path.append(_p)

import numpy as np

NUM_AUTHOR = 131072
D = 128
N_NODES = 32768
G = 32
NCORES = 8
NPC = N_NODES // NCORES   # 4096
P = 128
TILES = NPC // P          # 32
NCH = 4                   # table chunks (int16 idx limit: 32768 rows)
CH = NUM_AUTHOR // NCH    # 32768
NG = 8                    # tile groups (gather granularity)
TPG = TILES // NG         # 8 tiles per group
PAD_ID = 255

_CACHE = {}
LAST_RESULT = None


def _sort_cores(neighbors, lengths):
    """Per-core degree sort + per-(tile, chunk) packed (idx, id) pairs."""
    neighbors = np.asarray(neighbors).reshape(NCORES, NPC, G)
    lengths = np.asarray(lengths).reshape(NCORES, NPC)
    s2dt = np.array([(s % NG) * TPG + s // NG for s in range(TILES)])
    dt2s = np.argsort(s2dt)
    cores = []
    for ci in range(NCORES):
        order = np.argsort(-lengths[ci], kind="stable")
        nb, ln = neighbors[ci][order], lengths[ci][order]
        device_order = np.empty(NPC, dtype=np.int64)
        scl = np.zeros((P, TILES), dtype=np.float32)
        pairs = {}
        for dt in range(TILES):
            s = int(dt2s[dt])
            rows = slice(s * P, (s + 1) * P)
            device_order[dt * P:(dt + 1) * P] = order[rows]
            l = ln[rows]
            scl[:, dt] = np.where(l > 0, 1.0 / np.maximum(l, 1), 0.0)
            nbt = nb[rows]
            mask = np.arange(G)[None, :] < l[:, None]
            for c in range(NCH):
                m = mask & (nbt // CH == c)
                pp, jj = np.nonzero(m)
                idxs = (nbt[pp, jj] - c * CH).astype(np.int16)
                o = np.argsort(idxs, kind="stable")
                pairs[(dt, c)] = (idxs[o], pp[o].astype(np.int16))
        cores.append(dict(device_order=device_order, scl=scl, pairs=pairs))
    return cores


def _layout(cores):
    """Common (max-over-cores) block structure + buffer offsets."""
    nblk = {}
    for dt in range(TILES):
        for c in range(NCH):
            mx = max((len(co["pairs"][(dt, c)][0]) + P - 1) // P
                     for co in cores)
            nblk[(dt, c)] = mx
        if sum(nblk[(dt, c)] for c in range(NCH)) == 0:
            nblk[(dt, NCH - 1)] = 1

    calls = []          # (g, c, num_idxs, idx_col_off, gbuf_blk_off)
    tile_blocks = {dt: [] for dt in range(TILES)}   # gbuf-local blk ids
    gblocks = [0] * NG
    idx_off = 0
    for g in range(NG):
        lb = 0          # group-local block offset
        for c in range(NCH):
            n = 0
            for dt in range(g * TPG, (g + 1) * TPG):
                k = nblk[(dt, c)]
                if k == 0:
                    continue
                for b in range(k):
                    tile_blocks[dt].append(lb + b)
                n += k * P
                lb += k
            calls.append((g, c, n, idx_off, lb - n // P))
            idx_off += n // 16
        gblocks[g] = lb
    # ids array is tile-major: tile dt's blocks at cols tile_off[dt]..+nb
    tile_off = {}
    o = 0
    for dt in range(TILES):
        tile_off[dt] = o
        o += len(tile_blocks[dt])
    return dict(nblk=nblk, calls=calls, tile_blocks=tile_blocks,
                gblocks=gblocks, tile_off=tile_off, idx_w=idx_off, nblocks=o)


def _fill_inputs(cores, lay, a2e_bf):
    """Per-core DRAM images: wrapped idx (int16), ids (bf16), scl, iota."""
    import ml_dtypes
    bf16 = np.float16
    iw = lay["idx_w"]
    nblocks = lay["nblocks"]
    iota = np.broadcast_to(np.arange(P, dtype=np.float32), (P, P)).astype(bf16)
    in_maps = []
    for co in cores:
        idx_dram = np.zeros((P, iw), np.int16)
        ids_dram = np.full((P, nblocks), PAD_ID, np.float32)
        for (g, c, n, ioff, _gb) in lay["calls"]:
            if n == 0:
                continue
            si = []
            for dt in range(g * TPG, (g + 1) * TPG):
                k = lay["nblk"][(dt, c)]
                if k == 0:
                    continue
                ti, _tp = co["pairs"][(dt, c)]
                pad = k * P - len(ti)
                fill = ti[-1] if len(ti) else np.int16(0)
                si.append(np.concatenate(
                    [ti, np.full(pad, fill, np.int16)]))
            si = np.concatenate(si)
            assert len(si) == n
            w = si.reshape(n // 16, 16).T          # [16, n/16] wrapped
            idx_dram[:, ioff:ioff + n // 16] = np.tile(w, (8, 1))
        # ids tile-major: tile dt's j-th block (c asc, stream order) at
        # col tile_off[dt] + j — matches the DVE is_equal read
        for dt in range(TILES):
            col = lay["tile_off"][dt]
            for c in range(NCH):
                k = lay["nblk"][(dt, c)]
                if k == 0:
                    continue
                _ti, tp = co["pairs"][(dt, c)]
                pad = k * P - len(tp)
                tp = np.concatenate([tp, np.full(pad, PAD_ID, np.int16)])
                ids_dram[:, col:col + k] = tp.reshape(k, P).T
                col += k
        in_maps.append({
            "idx": idx_dram,
            "ids": ids_dram.astype(bf16),
            "iota": iota,
            "scl": co["scl"],
            "a2e": a2e_bf,
        })
    return in_maps


def _build_program(lay):
    from concourse import bacc, bass, mybir
    from concourse.library_config import mlp

    nc = bacc.Bacc("TRN2", target_bir_lowering=False, debug=False,
                   enable_asserts=False, num_devices=NCORES,
                   num_swdge_queues=4)
    dt_ = mybir.dt
    calls = lay["calls"]
    gblocks = lay["gblocks"]
    tile_blocks = lay["tile_blocks"]
    nblocks = lay["nblocks"]
    iw = lay["idx_w"]
    gmax = [max(gblocks[g] for g in range(p, NG, 2)) for p in (0, 1)]
    tbmax = [max(len(tile_blocks[dt]) for dt in range(p, TILES, 4))
             for p in (0, 1, 2, 3)]

    a2e = nc.dram_tensor("a2e", [NUM_AUTHOR, D], dt_.float16,
                         kind="ExternalInput")
    idx = nc.dram_tensor("idx", [P, iw], dt_.int16, kind="ExternalInput")
    ids = nc.dram_tensor("ids", [P, nblocks], dt_.float16,
                         kind="ExternalInput")
    iota = nc.dram_tensor("iota", [P, P], dt_.float16, kind="ExternalInput")
    scl = nc.dram_tensor("scl", [P, TILES], dt_.float32, kind="ExternalInput")
    out = nc.dram_tensor("out", [NPC, D], dt_.float32, kind="ExternalOutput")

    from contextlib import ExitStack
    with ExitStack() as stack:
        block = stack.enter_context(nc.Block())
        idx_sb = stack.enter_context(nc.sbuf_tensor("idx_sb", [P, iw], dt_.int16))
        ids_sb = stack.enter_context(nc.sbuf_tensor("ids_sb", [P, nblocks], dt_.float16))
        iota_sb = stack.enter_context(nc.sbuf_tensor("iota_sb", [P, P], dt_.float16))
        scl_sb = stack.enter_context(nc.sbuf_tensor("scl_sb", [P, TILES], dt_.float32))
        gb0 = stack.enter_context(nc.sbuf_tensor("gb0", [P, max(gmax[0], 1) * D], dt_.float16))
        gb1 = stack.enter_context(nc.sbuf_tensor("gb1", [P, max(gmax[1], 1) * D], dt_.float16))
        sb0 = stack.enter_context(nc.sbuf_tensor("sb0", [P, max(tbmax[0], 1) * D], dt_.float16))
        sb1 = stack.enter_context(nc.sbuf_tensor("sb1", [P, max(tbmax[1], 1) * D], dt_.float16))
        sb2 = stack.enter_context(nc.sbuf_tensor("sb2", [P, max(tbmax[2], 1) * D], dt_.float16))
        sb3 = stack.enter_context(nc.sbuf_tensor("sb3", [P, max(tbmax[3], 1) * D], dt_.float16))
        ob0 = stack.enter_context(nc.sbuf_tensor("ob0", [P, D], dt_.float32))
        ob1 = stack.enter_context(nc.sbuf_tensor("ob1", [P, D], dt_.float32))
        ps0 = stack.enter_context(nc.psum_tensor("ps0", [P, D], dt_.float32))
        ps1 = stack.enter_context(nc.psum_tensor("ps1", [P, D], dt_.float32))
        ps2 = stack.enter_context(nc.psum_tensor("ps2", [P, D], dt_.float32))
        ps3 = stack.enter_context(nc.psum_tensor("ps3", [P, D], dt_.float32))
        iosem = stack.enter_context(nc.semaphore("iosem"))
        idxsem = stack.enter_context(nc.semaphore("idxsem"))
        sclsem = stack.enter_context(nc.semaphore("sclsem"))
        dsem0 = stack.enter_context(nc.semaphore("dsem0"))
        dsem1 = stack.enter_context(nc.semaphore("dsem1"))
        ssem = stack.enter_context(nc.semaphore("ssem"))
        pesem = stack.enter_context(nc.semaphore("pesem"))
        asem = stack.enter_context(nc.semaphore("asem"))
        wsem0 = stack.enter_context(nc.semaphore("wsem0"))
        wsem1 = stack.enter_context(nc.semaphore("wsem1"))
        gbuf = [gb0, gb1]
        sbuf_ = [sb0, sb1, sb2, sb3]
        obuf = [ob0, ob1]
        psum = [ps0, ps1, ps2, ps3]
        dsem = [dsem0, dsem1]
        wsem = [wsem0, wsem1]
        # per-parity dsem target counts after each group's calls
        dcnt = {0: 0, 1: 0}
        dtarget = {}
        for g in range(NG):
            dcnt[g % 2] += 16 * sum(1 for (gg, c, n, io, gb) in calls
                                    if gg == g and n > 0)
            dtarget[g] = dcnt[g % 2]

        @block.sync
        def _(sync):
            sync.dma_start(out=idx_sb[:], in_=idx[:]).then_inc(idxsem, 16)
            sync.dma_start(out=ids_sb[:], in_=ids[:]).then_inc(iosem, 16)
            sync.dma_start(out=iota_sb[:], in_=iota[:]).then_inc(iosem, 16)
            sync.dma_start(out=scl_sb[:], in_=scl[:]).then_inc(sclsem, 16)
            for dt in range(TILES):
                sync.wait_ge(asem, dt + 1)
                sync.dma_start(
                    out=out[dt * P:(dt + 1) * P, :], in_=obuf[dt % 2][:]
                ).then_inc(wsem[dt % 2], 16)
            sync.wait_ge(wsem0, 16 * (TILES // 2))
            sync.wait_ge(wsem1, 16 * (TILES // 2))

        @block.gpsimd
        def _(gpsimd):
            gpsimd.load_library(mlp)
            gpsimd.wait_ge(idxsem, 16)
            for g in range(NG):
                if g >= 2:
                    gpsimd.wait_ge(pesem, (g - 1) * TPG)  # gbuf parity free
                for (gg, c, n, ioff, gboff) in calls:
                    if gg != g or n == 0:
                        continue
                    gpsimd.dma_gather(
                        out_ap=(gbuf[g % 2][:, gboff * D:(gboff + n // P) * D]
                                .rearrange("p (b d) -> p b d",
                                           b=n // P, d=D)),
                        in_ap=a2e[c * CH:(c + 1) * CH, :],
                        idxs_ap=idx_sb[:, ioff:ioff + n // 16],
                        num_idxs=n,
                        num_idxs_reg=n,
                        elem_size=D,
                        single_packet=False,
                        queue_num=c,
                    ).then_inc(dsem[g % 2], 16)

        @block.vector
        def _(vector):
            vector.wait_ge(iosem, 32)
            for dt in range(TILES):
                if dt >= 4:
                    vector.wait_ge(pesem, dt - 3)  # S ring slot free
                nb = len(tile_blocks[dt])
                b0 = lay["tile_off"][dt]           # ids col offset (tile-major)
                sv = (sbuf_[dt % 4][:, 0:nb * D]
                      .rearrange("p (b d) -> p b d", b=nb, d=D))
                idv = (ids_sb[:, b0:b0 + nb]
                       .rearrange("p (b o) -> p b o", o=1)
                       .broadcast_to([P, nb, P]))
                iov = (iota_sb[:].rearrange("p (o d) -> p o d", o=1)
                       .broadcast_to([P, nb, P]))
                vector.tensor_tensor(
                    out=sv, in0=idv, in1=iov, op=mybir.AluOpType.is_equal,
                ).then_inc(ssem, 1)

        @block.tensor
        def _(tensor):
            for dt in range(TILES):
                g = dt // TPG
                tensor.wait_ge(dsem[g % 2], dtarget[g])
                tensor.wait_ge(ssem, dt + 1)
                if dt >= 4:
                    tensor.wait_ge(asem, dt - 3)   # psum bank free
                blocks = tile_blocks[dt]
                last = len(blocks) - 1
                for j, lb in enumerate(blocks):
                    mm = tensor.matmul(
                        psum[dt % 4][:],
                        lhsT=(sbuf_[dt % 4][:, j * D:(j + 1) * D]),
                        rhs=(gbuf[g % 2][:, lb * D:(lb + 1) * D]),
                        start=(j == 0),
                        stop=(j == last),
                    )
                    if j == last:
                        mm.then_inc(pesem, 1)

        @block.scalar
        def _(scalar):
            from concourse import mybir as mb
            scalar.wait_ge(sclsem, 16)
            for dt in range(TILES):
                scalar.wait_ge(pesem, dt + 1)
                if dt >= 2:
                    scalar.wait_ge(wsem[dt % 2], 16 * (dt // 2))
                scalar.activation(
                    out=obuf[dt % 2][:],
                    in_=psum[dt % 4][:],
                    func=mb.ActivationFunctionType.Copy,
                    scale=scl_sb[:, dt:dt + 1],
                ).then_inc(asem, 1)

    nc.compile()
    return nc


def _install_ntff_hook_shim():
    import types
    if "antenv.axon_hooks" in sys.modules:
        return
    from trn_agent_boot.trn_boot import _ntff_profile_via_ctypes
    hook = _ntff_profile_via_ctypes("/opt/axon/libaxon_pjrt.so")
    mod = types.ModuleType("antenv.axon_hooks")
    mod._hook = hook
    mod.get_axon_ntff_profile_hook = lambda: mod._hook
    mod.set_axon_ntff_profile_hook = lambda h: setattr(mod, "_hook", h)
    sys.modules["antenv.axon_hooks"] = mod


def kernel(node, neighbors, lengths, a2e, _trace=False):
    global LAST_RESULT
    import ml_dtypes
    from concourse.bass_utils import run_bass_kernel_spmd

    if _trace:
        try:
            _install_ntff_hook_shim()
            import concourse.bass_utils as _bu
            _bu.upload_artifacts = lambda tmpdir: f"local://{tmpdir}"
        except Exception as e:
            print(f"ntff hook shim failed ({e}); running without trace")
            _trace = False

    cores = _sort_cores(neighbors, lengths)
    lay = _layout(cores)
    key = (tuple(lay["calls"]),
           tuple(tuple(lay["tile_blocks"][dt]) for dt in range(TILES)))
    if _CACHE.get("key") != key:
        _CACHE["nc"] = _build_program(lay)
        _CACHE["key"] = key
    nc = _CACHE["nc"]

    a2e_bf = np.asarray(a2e, dtype=np.float32).astype(np.float16)
    in_maps = _fill_inputs(cores, lay, a2e_bf)
    res = run_bass_kernel_spmd(nc, in_maps, list(range(NCORES)), trace=_trace)
    LAST_RESULT = res

    final = np.empty((N_NODES, D), dtype=np.float32)
    for ci in range(NCORES):
        blockv = final[ci * NPC:(ci + 1) * NPC]
        blockv[cores[ci]["device_order"]] = res.results[ci]["out"]
    return final


# revision 9
# speedup vs baseline: 3.8296x; 1.0191x over previous
"""v4: packed bf16 dma_gather + PE one-hot segment-sum.

Per core (4096 nodes, table replicated): nodes sorted by degree desc into 32
tiles of 128; tiles dealt round-robin into 4 groups (balanced bytes). Per
(group, table-chunk) the valid (idx, node) pairs of its 8 tiles are packed
into a contiguous stream (idx-sorted per tile, padded to x128, pad id=255)
and fetched with ONE dma_gather (int16 local idx, 16-partition wrap).
Gathered slot k lands at partition k%128, block k//128 — each 128-slot block
is a ready matmul rhs [k=128, d=128]. DVE builds the S^T one-hot [k, node]
per tile via is_equal(ids, iota); PE accumulates out[node, d] += S^T.T @ G
into PSUM; ACT copies PSUM->SBUF scaled by 1/len; sync stores per tile.

Why: the baseline's 528 indirect_dma_start/core each pay ~1us SWDGE fixed
cost (~570us serial descriptor generation). dma_gather amortizes it
(~num_idxs/16 descriptors, 16 calls/core) and bf16 halves HBM bytes. The
block structure is the max over cores so one SPMD program serves all 8.
"""
import os
import sys

for _p in ("/opt/trn_rl_repo", "/opt/pypackages"):
    if _p not in sys.path and os.path.isdir(_p):
        sys.path.append(_p)

import numpy as np

NUM_AUTHOR = 131072
D = 128
N_NODES = 32768
G = 32
NCORES = 8
NPC = N_NODES // NCORES   # 4096
P = 128
TILES = NPC // P          # 32
NCH = 4                   # table chunks (int16 idx limit: 32768 rows)
CH = NUM_AUTHOR // NCH    # 32768
NG = 8                    # tile groups (gather granularity)
TPG = TILES // NG         # 8 tiles per group
PAD_ID = 255

_CACHE = {}
LAST_RESULT = None


def _sort_cores(neighbors, lengths):
    """Per-core degree sort + per-(tile, chunk) packed (idx, id) pairs."""
    neighbors = np.asarray(neighbors).reshape(NCORES, NPC, G)
    lengths = np.asarray(lengths).reshape(NCORES, NPC)
    s2dt = np.array([(s % NG) * TPG + s // NG for s in range(TILES)])
    dt2s = np.argsort(s2dt)
    cores = []
    for ci in range(NCORES):
        order = np.argsort(-lengths[ci], kind="stable")
        nb, ln = neighbors[ci][order], lengths[ci][order]
        device_order = np.empty(NPC, dtype=np.int64)
        scl = np.zeros((P, TILES), dtype=np.float32)
        pairs = {}
        for dt in range(TILES):
            s = int(dt2s[dt])
            rows = slice(s * P, (s + 1) * P)
            device_order[dt * P:(dt + 1) * P] = order[rows]
            l = ln[rows]
            scl[:, dt] = np.where(l > 0, 1.0 / np.maximum(l, 1), 0.0)
            nbt = nb[rows]
            mask = np.arange(G)[None, :] < l[:, None]
            for c in range(NCH):
                m = mask & (nbt // CH == c)
                pp, jj = np.nonzero(m)
                idxs = (nbt[pp, jj] - c * CH).astype(np.int16)
                o = np.argsort(idxs, kind="stable")
                pairs[(dt, c)] = (idxs[o], pp[o].astype(np.int16))
        cores.append(dict(device_order=device_order, scl=scl, pairs=pairs))
    return cores


def _layout(cores):
    """Common (max-over-cores) slot counts; blocks may span tiles.

    Per (dt, c) the common slot count vbar is the max over cores (cores with
    fewer pad with repeats, pad id). Streams concat per-(dt, c) runs and are
    rounded to x128 at the END only. tile_blocks[dt] lists (local block,
    ids col) for every block the tile's run overlaps; boundary blocks get
    their own ids column per overlapping tile.
    """
    vbar = {}
    for dt in range(TILES):
        for c in range(NCH):
            vbar[(dt, c)] = max(len(co["pairs"][(dt, c)][0]) for co in cores)
        if sum(vbar[(dt, c)] for c in range(NCH)) == 0:
            vbar[(dt, NCH - 1)] = 1

    calls = []          # (g, c, num_idxs, idx_col_off, gbuf_blk_off)
    tile_blocks = {dt: [] for dt in range(TILES)}   # (local blk, ids col)
    run_off = {}        # (dt, c) -> slot offset within its (g, c) stream
    gblocks = [0] * NG
    idx_off = 0
    ncol = 0
    for g in range(NG):
        lb0 = 0         # group-local block offset of this stream
        for c in range(NCH):
            o = 0
            for dt in range(g * TPG, (g + 1) * TPG):
                v = vbar[(dt, c)]
                if v == 0:
                    continue
                run_off[(dt, c)] = o
                b_lo = o // P
                b_hi = (o + v - 1) // P
                for b in range(b_lo, b_hi + 1):
                    tile_blocks[dt].append((lb0 + b, ncol))
                    ncol += 1
                o += v
            n = ((o + P - 1) // P) * P
            calls.append((g, c, n, idx_off, lb0))
            idx_off += n // 16
            lb0 += n // P
        gblocks[g] = lb0
    # ids col ranges per tile are assigned in tile_blocks order, but the
    # host fills them per (dt, c, block); record per-tile col start
    tile_off = {}
    o = 0
    for dt in range(TILES):
        tile_off[dt] = o
        o += len(tile_blocks[dt])
    return dict(vbar=vbar, calls=calls, tile_blocks=tile_blocks,
                run_off=run_off, gblocks=gblocks, tile_off=tile_off,
                idx_w=idx_off, nblocks=o)


def _fill_inputs(cores, lay, a2e_bf):
    """Per-core DRAM images: wrapped idx (int16), ids (bf16), scl, iota."""
    import ml_dtypes
    bf16 = np.float16
    iw = lay["idx_w"]
    nblocks = lay["nblocks"]
    iota = np.broadcast_to(np.arange(P, dtype=np.float32), (P, P)).astype(bf16)
    in_maps = []
    for co in cores:
        idx_dram = np.zeros((P, iw), np.int16)
        ids_dram = np.full((P, nblocks), PAD_ID, np.float32)
        for (g, c, n, ioff, _gb) in lay["calls"]:
            if n == 0:
                continue
            si = []
            for dt in range(g * TPG, (g + 1) * TPG):
                v = lay["vbar"][(dt, c)]
                if v == 0:
                    continue
                ti, _tp = co["pairs"][(dt, c)]
                pad = v - len(ti)
                fill = ti[-1] if len(ti) else np.int16(0)
                si.append(np.concatenate(
                    [ti, np.full(pad, fill, np.int16)]))
            si = np.concatenate(si)
            tailpad = n - len(si)
            if tailpad:
                si = np.concatenate([si, np.full(tailpad, si[-1], np.int16)])
            w = si.reshape(n // 16, 16).T          # [16, n/16] wrapped
            idx_dram[:, ioff:ioff + n // 16] = np.tile(w, (8, 1))
        # ids tile-major: one col per (tile, overlapped block); slots outside
        # the tile's run (or beyond this core's valid count) get PAD_ID
        for dt in range(TILES):
            col = lay["tile_off"][dt]
            for c in range(NCH):
                v = lay["vbar"][(dt, c)]
                if v == 0:
                    continue
                _ti, tp = co["pairs"][(dt, c)]
                o = lay["run_off"][(dt, c)]
                full = np.full(v, PAD_ID, np.int16)
                full[:len(tp)] = tp
                b_lo = o // P
                b_hi = (o + v - 1) // P
                for b in range(b_lo, b_hi + 1):
                    colv = np.full(P, PAD_ID, np.int16)
                    s0 = max(o, b * P)
                    s1 = min(o + v, (b + 1) * P)
                    colv[s0 - b * P:s1 - b * P] = full[s0 - o:s1 - o]
                    ids_dram[:, col] = colv
                    col += 1
        in_maps.append({
            "idx": idx_dram,
            "ids": ids_dram.astype(bf16),
            "iota": iota,
            "scl": co["scl"],
            "a2e": a2e_bf,
        })
    return in_maps


def _build_program(lay):
    from concourse import bacc, bass, mybir
    from concourse.library_config import mlp

    nc = bacc.Bacc("TRN2", target_bir_lowering=False, debug=False,
                   enable_asserts=False, num_devices=NCORES,
                   num_swdge_queues=4)
    dt_ = mybir.dt
    calls = lay["calls"]
    gblocks = lay["gblocks"]
    tile_blocks = lay["tile_blocks"]
    nblocks = lay["nblocks"]
    iw = lay["idx_w"]
    gmax = [max(gblocks[g] for g in range(p, NG, 2)) for p in (0, 1)]
    tbmax = [max(len(tile_blocks[dt]) for dt in range(p, TILES, 4))
             for p in (0, 1, 2, 3)]

    a2e = nc.dram_tensor("a2e", [NUM_AUTHOR, D], dt_.float16,
                         kind="ExternalInput")
    idx = nc.dram_tensor("idx", [P, iw], dt_.int16, kind="ExternalInput")
    ids = nc.dram_tensor("ids", [P, nblocks], dt_.float16,
                         kind="ExternalInput")
    iota = nc.dram_tensor("iota", [P, P], dt_.float16, kind="ExternalInput")
    scl = nc.dram_tensor("scl", [P, TILES], dt_.float32, kind="ExternalInput")
    out = nc.dram_tensor("out", [NPC, D], dt_.float32, kind="ExternalOutput")

    from contextlib import ExitStack
    with ExitStack() as stack:
        block = stack.enter_context(nc.Block())
        idx_sb = stack.enter_context(nc.sbuf_tensor("idx_sb", [P, iw], dt_.int16))
        ids_sb = stack.enter_context(nc.sbuf_tensor("ids_sb", [P, nblocks], dt_.float16))
        iota_sb = stack.enter_context(nc.sbuf_tensor("iota_sb", [P, P], dt_.float16))
        scl_sb = stack.enter_context(nc.sbuf_tensor("scl_sb", [P, TILES], dt_.float32))
        gb0 = stack.enter_context(nc.sbuf_tensor("gb0", [P, max(gmax[0], 1) * D], dt_.float16))
        gb1 = stack.enter_context(nc.sbuf_tensor("gb1", [P, max(gmax[1], 1) * D], dt_.float16))
        sb0 = stack.enter_context(nc.sbuf_tensor("sb0", [P, max(tbmax[0], 1) * D], dt_.float16))
        sb1 = stack.enter_context(nc.sbuf_tensor("sb1", [P, max(tbmax[1], 1) * D], dt_.float16))
        sb2 = stack.enter_context(nc.sbuf_tensor("sb2", [P, max(tbmax[2], 1) * D], dt_.float16))
        sb3 = stack.enter_context(nc.sbuf_tensor("sb3", [P, max(tbmax[3], 1) * D], dt_.float16))
        sb4 = stack.enter_context(nc.sbuf_tensor("sb4", [P, max(tbmax[0], 1) * D], dt_.float16))
        sb5 = stack.enter_context(nc.sbuf_tensor("sb5", [P, max(tbmax[1], 1) * D], dt_.float16))
        sb6 = stack.enter_context(nc.sbuf_tensor("sb6", [P, max(tbmax[2], 1) * D], dt_.float16))
        sb7 = stack.enter_context(nc.sbuf_tensor("sb7", [P, max(tbmax[3], 1) * D], dt_.float16))
        ob0 = stack.enter_context(nc.sbuf_tensor("ob0", [P, D], dt_.float32))
        ob1 = stack.enter_context(nc.sbuf_tensor("ob1", [P, D], dt_.float32))
        ps0 = stack.enter_context(nc.psum_tensor("ps0", [P, D], dt_.float32))
        ps1 = stack.enter_context(nc.psum_tensor("ps1", [P, D], dt_.float32))
        ps2 = stack.enter_context(nc.psum_tensor("ps2", [P, D], dt_.float32))
        ps3 = stack.enter_context(nc.psum_tensor("ps3", [P, D], dt_.float32))
        iosem = stack.enter_context(nc.semaphore("iosem"))
        idxsem = stack.enter_context(nc.semaphore("idxsem"))
        sclsem = stack.enter_context(nc.semaphore("sclsem"))
        dsem0 = stack.enter_context(nc.semaphore("dsem0"))
        dsem1 = stack.enter_context(nc.semaphore("dsem1"))
        ssem = stack.enter_context(nc.semaphore("ssem"))
        pesem = stack.enter_context(nc.semaphore("pesem"))
        asem = stack.enter_context(nc.semaphore("asem"))
        wsem0 = stack.enter_context(nc.semaphore("wsem0"))
        wsem1 = stack.enter_context(nc.semaphore("wsem1"))
        gbuf = [gb0, gb1]
        sbuf_ = [sb0, sb1, sb2, sb3, sb4, sb5, sb6, sb7]
        obuf = [ob0, ob1]
        psum = [ps0, ps1, ps2, ps3]
        dsem = [dsem0, dsem1]
        wsem = [wsem0, wsem1]
        # per-parity dsem target counts after each group's calls
        dcnt = {0: 0, 1: 0}
        dtarget = {}
        for g in range(NG):
            dcnt[g % 2] += 16 * sum(1 for (gg, c, n, io, gb) in calls
                                    if gg == g and n > 0)
            dtarget[g] = dcnt[g % 2]

        @block.sync
        def _(sync):
            g0w = max(n // 16 for (gg, c, n, io, gb) in calls if gg == 0)
            g0end = max(io + n // 16 for (gg, c, n, io, gb) in calls
                        if gg == 0)
            sync.dma_start(out=idx_sb[:, 0:g0end],
                           in_=idx[:, 0:g0end]).then_inc(idxsem, 16)
            sync.dma_start(out=idx_sb[:, g0end:iw],
                           in_=idx[:, g0end:iw]).then_inc(idxsem, 16)
            sync.dma_start(out=ids_sb[:], in_=ids[:]).then_inc(iosem, 16)
            sync.dma_start(out=iota_sb[:], in_=iota[:]).then_inc(iosem, 16)
            sync.dma_start(out=scl_sb[:], in_=scl[:]).then_inc(sclsem, 16)
            for dt in range(TILES):
                sync.wait_ge(asem, dt + 1)
                sync.dma_start(
                    out=out[dt * P:(dt + 1) * P, :], in_=obuf[dt % 2][:]
                ).then_inc(wsem[dt % 2], 16)
            sync.wait_ge(wsem0, 16 * (TILES // 2))
            sync.wait_ge(wsem1, 16 * (TILES // 2))

        @block.gpsimd
        def _(gpsimd):
            gpsimd.load_library(mlp)
            gpsimd.wait_ge(idxsem, 16)
            for g in range(NG):
                if g == 1:
                    gpsimd.wait_ge(idxsem, 32)
                if g >= 2:
                    gpsimd.wait_ge(pesem, (g - 1) * TPG)  # gbuf parity free
                for (gg, c, n, ioff, gboff) in calls:
                    if gg != g or n == 0:
                        continue
                    gpsimd.dma_gather(
                        out_ap=(gbuf[g % 2][:, gboff * D:(gboff + n // P) * D]
                                .rearrange("p (b d) -> p b d",
                                           b=n // P, d=D)),
                        in_ap=a2e[c * CH:(c + 1) * CH, :],
                        idxs_ap=idx_sb[:, ioff:ioff + n // 16],
                        num_idxs=n,
                        num_idxs_reg=n,
                        elem_size=D,
                        single_packet=False,
                        queue_num=c,
                    ).then_inc(dsem[g % 2], 16)

        @block.vector
        def _(vector):
            vector.wait_ge(iosem, 32)
            for dt in range(TILES):
                if dt >= 8:
                    vector.wait_ge(pesem, dt - 7)  # S ring slot free
                nb = len(tile_blocks[dt])
                b0 = lay["tile_off"][dt]           # ids col offset (tile-major)
                sv = (sbuf_[dt % 8][:, 0:nb * D]
                      .rearrange("p (b d) -> p b d", b=nb, d=D))
                idv = (ids_sb[:, b0:b0 + nb]
                       .rearrange("p (b o) -> p b o", o=1)
                       .broadcast_to([P, nb, P]))
                iov = (iota_sb[:].rearrange("p (o d) -> p o d", o=1)
                       .broadcast_to([P, nb, P]))
                vector.tensor_tensor(
                    out=sv, in0=idv, in1=iov, op=mybir.AluOpType.is_equal,
                ).then_inc(ssem, 1)

        @block.tensor
        def _(tensor):
            for dt in range(TILES):
                g = dt // TPG
                tensor.wait_ge(dsem[g % 2], dtarget[g])
                tensor.wait_ge(ssem, dt + 1)
                if dt >= 4:
                    tensor.wait_ge(asem, dt - 3)   # psum bank free
                blocks = [lb for (lb, _ic) in tile_blocks[dt]]
                last = len(blocks) - 1
                for j, lb in enumerate(blocks):
                    mm = tensor.matmul(
                        psum[dt % 4][:],
                        lhsT=(sbuf_[dt % 8][:, j * D:(j + 1) * D]),
                        rhs=(gbuf[g % 2][:, lb * D:(lb + 1) * D]),
                        start=(j == 0),
                        stop=(j == last),
                    )
                    if j == last:
                        mm.then_inc(pesem, 1)

        @block.scalar
        def _(scalar):
            from concourse import mybir as mb
            scalar.wait_ge(sclsem, 16)
            for dt in range(TILES):
                scalar.wait_ge(pesem, dt + 1)
                if dt >= 2:
                    scalar.wait_ge(wsem[dt % 2], 16 * (dt // 2))
                scalar.activation(
                    out=obuf[dt % 2][:],
                    in_=psum[dt % 4][:],
                    func=mb.ActivationFunctionType.Copy,
                    scale=scl_sb[:, dt:dt + 1],
                ).then_inc(asem, 1)

    nc.compile()
    return nc


def _install_ntff_hook_shim():
    import types
    if "antenv.axon_hooks" in sys.modules:
        return
    from trn_agent_boot.trn_boot import _ntff_profile_via_ctypes
    hook = _ntff_profile_via_ctypes("/opt/axon/libaxon_pjrt.so")
    mod = types.ModuleType("antenv.axon_hooks")
    mod._hook = hook
    mod.get_axon_ntff_profile_hook = lambda: mod._hook
    mod.set_axon_ntff_profile_hook = lambda h: setattr(mod, "_hook", h)
    sys.modules["antenv.axon_hooks"] = mod


def kernel(node, neighbors, lengths, a2e, _trace=False):
    global LAST_RESULT
    import ml_dtypes
    from concourse.bass_utils import run_bass_kernel_spmd

    if _trace:
        try:
            _install_ntff_hook_shim()
            import concourse.bass_utils as _bu
            _bu.upload_artifacts = lambda tmpdir: f"local://{tmpdir}"
        except Exception as e:
            print(f"ntff hook shim failed ({e}); running without trace")
            _trace = False

    cores = _sort_cores(neighbors, lengths)
    lay = _layout(cores)
    key = (tuple(lay["calls"]),
           tuple(tuple(lay["tile_blocks"][dt]) for dt in range(TILES)))
    if _CACHE.get("key") != key:
        _CACHE["nc"] = _build_program(lay)
        _CACHE["key"] = key
    nc = _CACHE["nc"]

    a2e_bf = np.asarray(a2e, dtype=np.float32).astype(np.float16)
    in_maps = _fill_inputs(cores, lay, a2e_bf)
    res = run_bass_kernel_spmd(nc, in_maps, list(range(NCORES)), trace=_trace)
    LAST_RESULT = res

    final = np.empty((N_NODES, D), dtype=np.float32)
    for ci in range(NCORES):
        blockv = final[ci * NPC:(ci + 1) * NPC]
        blockv[cores[ci]["device_order"]] = res.results[ci]["out"]
    return final
